# revision 14
# baseline (speedup 1.0000x reference)
"""Causal multi-head self-attention on 8 Trainium2 NeuronCores.

Problem: B=8, T=1024, D=1024, 16 heads (H=64), fp32 in/out, causal softmax,
y = softmax(mask(q k^T)/sqrt(H)) v, then output projection. Weights are
nn.Linear style: q = x @ Wq^T etc.

Sharding: pure data-parallel - one batch element per core, weights
replicated, no collectives.

Per-core layout (all feature-major, zero on-device transposes); matmul
operands stored in bf16 (fp32 PSUM accumulation), fp32 output:
  host sends xT = x[b].T  [d, t]  and W*T = W*.T  [d_in, d_out]
  v-projection first, then per head pair pr the q/k projections for the NEXT
  pair are interleaved into the attention loop: the projection matmuls have
  no Activation-engine dependency, so they fill the PE bubbles created by
  the exp/mask latency chain.
  qT[do,t] = sum_d WqT[d,do] * xT[d,t]   (lhsT=WqT, rhs=xT)
  kT       likewise
  v[t,do]  = sum_d xT[d,t]  * WvT[d,do]  (lhsT=xT,  rhs=WvT) -> natural layout
  per head pair (2p, 2p+1), per tq block of 512, per tk block of 128:
    S^T[tk,tq]   = sum_hd kT_h[hd,tk] qT_h[hd,tq]   (both heads into one
                   [128,1024] PSUM superblock, one 512-col half per head)
    E = exp(S^T/8)                                  (one ACT op per superblock)
    diag blocks:  E = affine_select(E, f - p - 128g >= 0, fill=0) on GpSimd
                  (causal mask as a zero-fill, no PE mask matmuls)
    outX[m,tq]   = sum_tk Vplus_h[tk,m] E_h[tk,tq], Vplus_h = [v_h | 1]
    outT_h       = outX[0:64] * recip(outX[64])     (flash-style denominator;
                   reciprocal_approx_fast batched 4 rows at a time, broadcast
                   across partitions via a DRAM-bounce DMA)
  y[t,do] = sum_d outT[d,t] * WoT[d,do]  (lhsT=outT, rhs=WoT; the first three
  column blocks accumulate k=0..6 early so only the k=7 term waits on the
  last attention pair)
"""

import numpy as np
from contextlib import ExitStack

N_CORES = 8
T = 1024
D = 1024
NH = 16
HD = 64  # head dim
P = 128
NT = D // P  # 8 tiles of 128 along d or t
NQ = 2       # tq tiles of 512
TQ = 512
SCALE = 1.0 / 8.0  # 1/sqrt(64)

# matmul compute dtype: "fp32" (exact, 1/4 rate), "fp32r" (TF32-like, full
# rate), or "bf16" (full rate, half DMA/SBUF traffic)
MM_MODE = "bf16"

_COMPILED = {}


def _build(nc, tile, mybir, mm_dt):
    """Emit the per-core Tile program into nc."""
    f32 = mybir.dt.float32
    Exp = mybir.ActivationFunctionType.Exp

    mdt = mm_dt  # storage dtype for matmul operands
    xT_d = nc.declare_dram_parameter("xT", [D, T], mdt, isOutput=False)
    wqT_d = nc.declare_dram_parameter("wqT", [D, D], mdt, isOutput=False)
    wkT_d = nc.declare_dram_parameter("wkT", [D, D], mdt, isOutput=False)
    wvT_d = nc.declare_dram_parameter("wvT", [D, D], mdt, isOutput=False)
    woT_d = nc.declare_dram_parameter("woT", [D, D], mdt, isOutput=False)
    y_d = nc.declare_dram_parameter("y", [T, D], f32, isOutput=True)

    nrm_d = nc.dram_tensor("nrm_scratch", [8, 4, TQ], f32)

    def mm(out, lhsT, rhs, start, stop):
        nc.tensor.matmul(out, lhsT, rhs, start=start, stop=stop)

    with ExitStack() as ctx:
        tc = ctx.enter_context(tile.TileContext(nc))

        # ---- pools (one scope; phases interleave) ----
        pqk = ctx.enter_context(tc.tile_pool(name="pqk", bufs=6))
        pv = ctx.enter_context(tc.tile_pool(name="pv", bufs=8))
        pwo = ctx.enter_context(tc.tile_pool(name="pwo", bufs=1))
        pxt = ctx.enter_context(tc.tile_pool(name="pxt", bufs=8))
        pwq = ctx.enter_context(tc.tile_pool(name="pwq", bufs=8))
        pwk = ctx.enter_context(tc.tile_pool(name="pwk", bufs=1))
        pout = ctx.enter_context(tc.tile_pool(name="pout", bufs=8))
        # psum: 3 superblock slots (2 banks each) + 2 attV slots = 8 banks
        pp_big = ctx.enter_context(
            tc.tile_pool(name="pp_big", bufs=3, space="PSUM")
        )
        pp_o = ctx.enter_context(tc.tile_pool(name="pp_o", bufs=2, space="PSUM"))

        # v-plus layout: head h at cols [65h .. 65h+63], ones col at 65h+64
        VP = HD + 1
        vp = [pv.tile([P, NH * VP], mdt, tag="vp", name=f"vp{i}") for i in range(NT)]
        wo_all = pwo.tile([P, NT * D], mdt, name="wo_all")
        outT = [pout.tile([P, T], mdt, tag="ot", name=f"outT{i}")
                for i in range(NT)]
        xT = [pxt.tile([P, T], mdt, tag="xt", name=f"xTs{i}")
              for i in range(NT)]
        wq_sb = [pwq.tile([P, D], mdt, tag="wq", name=f"wq{i}")
                 for i in range(NT)]
        wk_all = pwk.tile([P, NT * D], mdt, name="wk_all")

        # ---- input DMAs, dependency-first: v-projection runs first ----
        with tc.tile_pool(name="pwv", bufs=1) as pwv:
            wv_sb = [pwv.tile([P, D], mdt, tag="wv", name=f"wv{i}", bufs=8)
                     for i in range(NT)]
            for k in range(NT):
                nc.sync.dma_start(
                    out=wv_sb[k][:], in_=wvT_d[k * P : (k + 1) * P, :]
                )
                nc.sync.dma_start(
                    out=xT[k][:], in_=xT_d[k * P : (k + 1) * P, :]
                )
            for k in range(NT):
                nc.sync.dma_start(
                    out=wq_sb[k][:], in_=wqT_d[k * P : (k + 1) * P, :]
                )
            nc.sync.dma_start(
                out=wk_all.rearrange("p (k d) -> p k d", d=D),
                in_=wkT_d.rearrange("(k p) d -> p k d", p=P),
            )
            nc.sync.dma_start(
                out=wo_all.rearrange("p (k d) -> p k d", d=D),
                in_=woT_d.rearrange("(k p) d -> p k d", p=P),
            )
            # ones columns of v-plus (denominator rows for the attV matmul)
            for m in range(NT):
                ones_cols = vp[m].rearrange(
                    "p (h c) -> p h c", c=VP
                )[:, :, VP - 1]
                nc.gpsimd.memset(ones_cols, 1.0)

            for m in range(NT):  # v projection (natural [t, do] layout)
                ps = pp_big.tile([P, T], f32, tag="ps")
                for n in range(NQ):
                    for k in range(NT):
                        mm(ps[:, n * TQ : (n + 1) * TQ],
                           xT[k][:, m * P : (m + 1) * P],
                           wv_sb[k][:, n * TQ : (n + 1) * TQ],
                           start=(k == 0), stop=(k == NT - 1))
                vdst = vp[m].rearrange("p (h c) -> p h c", c=VP)[:, :, 0:HD]
                vsrc = ps.rearrange("p (h c) -> p h c", c=HD)
                nc.vector.tensor_copy(vdst, vsrc)

            def qk_proj(pr):
                """Project q and k for head pair pr -> fresh rotation tiles."""
                ps = pp_big.tile([P, T], f32, tag="ps")
                for n in range(NQ):
                    for k in range(NT):
                        mm(ps[:, n * TQ : (n + 1) * TQ],
                           wq_sb[k][:, pr * P : (pr + 1) * P],
                           xT[k][:, n * TQ : (n + 1) * TQ],
                           start=(k == 0), stop=(k == NT - 1))
                qt = pqk.tile([P, T], mdt, tag="qk", name=f"qTt{pr}")
                nc.vector.tensor_copy(qt[:], ps[:])
                ps = pp_big.tile([P, T], f32, tag="ps")
                for n in range(NQ):
                    for k in range(NT):
                        mm(ps[:, n * TQ : (n + 1) * TQ],
                           wk_all[:, k * D + pr * P : k * D + (pr + 1) * P],
                           xT[k][:, n * TQ : (n + 1) * TQ],
                           start=(k == 0), stop=(k == NT - 1))
                kt = pqk.tile([P, T], mdt, tag="qk", name=f"kTt{pr}")
                nc.vector.tensor_copy(kt[:], ps[:])
                return qt, kt

            # ---- attention, with next pair's projections as PE filler ----
            with (
                tc.tile_pool(name="pe", bufs=9) as pe,
                tc.tile_pool(name="pux", bufs=4) as pux,
                tc.tile_pool(name="pct", bufs=2) as pct,
                tc.tile_pool(name="pbc", bufs=4) as pbc,
            ):
                pending = []  # (ux, it, j, half) awaiting normalize
                state = {"flush_no": 0}

                def flush():
                    # denominator rows sit at partition 64 of each ux:
                    # gather to partitions 0..3, one batched fast
                    # reciprocal, broadcast back via a DRAM-bounce DMA
                    if not pending:
                        return
                    fno = state["flush_no"]
                    nb = len(pending)
                    ct = pct.tile([4, TQ], f32, tag="ct")
                    cr = pct.tile([4, TQ], f32, tag="cr")
                    for r, (ux, _, _, _) in enumerate(pending):
                        nc.sync.dma_start(
                            out=ct[r : r + 1, :], in_=ux[HD : HD + 1, :]
                        )
                    nc.vector.reciprocal_approx_fast(cr[:nb, :], ct[:nb, :])
                    nc.sync.dma_start(out=nrm_d[fno, :nb, :], in_=cr[:nb, :])
                    for r, (ux, it, j, half) in enumerate(pending):
                        bt = pbc.tile([HD, TQ], f32, tag="bt")
                        nc.sync.dma_start(
                            out=bt[:],
                            in_=nrm_d[fno, r : r + 1, :].to_broadcast([HD, TQ]),
                        )
                        if half == 0:
                            nc.vector.tensor_mul(
                                outT[it][0:HD, j * TQ : (j + 1) * TQ],
                                ux[0:HD, :], bt[:],
                            )
                        else:
                            nt_ = pbc.tile([HD, TQ], mdt, tag="nt")
                            nc.vector.tensor_mul(nt_[:], ux[0:HD, :], bt[:])
                            nc.sync.dma_start(
                                out=outT[it][HD:P, j * TQ : (j + 1) * TQ],
                                in_=nt_[:],
                            )
                    pending.clear()
                    state["flush_no"] = (fno + 1) % 8

                def scores(qt, kt, j, ni):
                    esup = []
                    for i in range(ni):
                        ps = pp_big.tile([P, 2 * TQ], f32, tag="ps")
                        g = i - 4 * j
                        for half in range(2):
                            po = half * HD
                            c = half * TQ
                            mm(ps[:, c : c + TQ],
                               kt[po : po + HD, i * P : (i + 1) * P],
                               qt[po : po + HD, j * TQ : (j + 1) * TQ],
                               start=True, stop=True)
                        e = pe.tile([P, 2 * TQ], mdt, tag="e")
                        nc.scalar.activation(e[:], ps[:], Exp, scale=SCALE)
                        if g >= 0:  # causal: keep where f - p - 128g >= 0
                            ev = e.rearrange("p (h f) -> p h f", h=2)
                            nc.gpsimd.affine_select(
                                ev, ev,
                                pattern=[[0, 2], [1, TQ]],
                                compare_op=mybir.AluOpType.is_ge,
                                fill=0.0,
                                base=-(P * g),
                                channel_multiplier=-1,
                            )
                        esup.append(e)
                    return esup

                def attv(esup, pr, j, ni):
                    for half in range(2):
                        h = 2 * pr + half
                        po_ps = pp_o.tile([HD + 1, TQ], f32)
                        for i in range(ni):
                            mm(po_ps[:],
                               vp[i][:, h * VP : h * VP + VP],
                               esup[i][:, half * TQ : (half + 1) * TQ],
                               start=(i == 0), stop=(i == ni - 1))
                        ux = pux.tile([HD + 1, TQ], f32, tag="ux")
                        nc.vector.tensor_copy(ux[:], po_ps[:])
                        pending.append((ux, pr, j, half))

                nxt = qk_proj(0)
                for pr in range(NH // 2):
                    qt, kt = nxt
                    e0 = scores(qt, kt, 0, 4)
                    if pr < NH // 2 - 1:
                        nxt = qk_proj(pr + 1)  # PE filler under exp/mask
                    attv(e0, pr, 0, 4)
                    e1 = scores(qt, kt, 1, 8)
                    attv(e1, pr, 1, 8)
                    flush()

        # ---- output projection ----
        # first NPRE column blocks accumulate k=0..6 early; only the k=7
        # term (and blocks 3..7) wait on the last attention pair's outT[7]
        NPRE = 3
        with tc.tile_pool(name="py", bufs=2) as py:
            pre = []
            for m in range(NPRE):
                ps = pp_big.tile([P, T], f32, tag="ps")
                for n in range(NQ):
                    for k in range(NT - 1):
                        mm(ps[:, n * TQ : (n + 1) * TQ],
                           outT[k][:, m * P : (m + 1) * P],
                           wo_all[:, k * D + n * TQ : k * D + (n + 1) * TQ],
                           start=(k == 0), stop=False)
                pre.append(ps)
            for m in range(NT):
                if m < NPRE:
                    ps = pre[m]
                    k0 = NT - 1
                else:
                    ps = pp_big.tile([P, T], f32, tag="ps")
                    k0 = 0
                for n in range(NQ):
                    for k in range(k0, NT):
                        mm(ps[:, n * TQ : (n + 1) * TQ],
                           outT[k][:, m * P : (m + 1) * P],
                           wo_all[:, k * D + n * TQ : k * D + (n + 1) * TQ],
                           start=(k == 0), stop=(k == NT - 1))
                ysb = py.tile([P, T], f32, tag="y")
                nc.vector.tensor_copy(ysb[:], ps[:])
                nc.sync.dma_start(
                    out=y_d[m * P : (m + 1) * P, :], in_=ysb[:]
                )
    return nc


def build_program(mm_mode=None):
    """Build + compile the SPMD program once; returns the Bacc object."""
    mode = mm_mode or MM_MODE
    if mode in _COMPILED:
        return _COMPILED[mode]
    import concourse.bacc as bacc
    import concourse.tile as tile
    from concourse import mybir

    mm_dt = {
        "fp32": mybir.dt.float32,
        "fp32r": mybir.dt.float32r,
        "bf16": mybir.dt.bfloat16,
    }[mode]
    nc = bacc.Bacc("TRN2", target_bir_lowering=False, debug=False,
                   num_devices=N_CORES)
    _build(nc, tile, mybir, mm_dt)
    nc.compile()
    _COMPILED[mode] = nc
    return nc


def _np_dt():
    if MM_MODE == "bf16":
        import ml_dtypes
        return ml_dtypes.bfloat16
    return np.float32


def make_in_maps(x, Wk, Wq, Wv, Wo):
    dt = _np_dt()
    wqT = np.ascontiguousarray(np.asarray(Wq, dtype=np.float32).T.astype(dt))
    wkT = np.ascontiguousarray(np.asarray(Wk, dtype=np.float32).T.astype(dt))
    wvT = np.ascontiguousarray(np.asarray(Wv, dtype=np.float32).T.astype(dt))
    woT = np.ascontiguousarray(np.asarray(Wo, dtype=np.float32).T.astype(dt))
    in_maps = []
    for b in range(N_CORES):
        in_maps.append({
            "xT": np.ascontiguousarray(x[b].T.astype(dt)),
            "wqT": wqT, "wkT": wkT, "wvT": wvT, "woT": woT,
        })
    return in_maps


def kernel(x, Wk, Wq, Wv, Wo):
    from concourse.bass_utils import run_bass_kernel_spmd

    x = np.asarray(x, dtype=np.float32)
    nc = build_program()
    in_maps = make_in_maps(x, Wk, Wq, Wv, Wo)
    res = run_bass_kernel_spmd(nc, in_maps, list(range(N_CORES)))
    return np.stack([res.results[c]["y"] for c in range(N_CORES)], axis=0)


# revision 15
# speedup vs baseline: 1.1492x; 1.1492x over previous
"""Causal multi-head self-attention on 8 Trainium2 NeuronCores.

Problem: B=8, T=1024, D=1024, 16 heads (H=64), fp32 in/out, causal softmax,
y = softmax(mask(q k^T)/sqrt(H)) v, then output projection. Weights are
nn.Linear style: q = x @ Wq^T etc.

Sharding: pure data-parallel - one batch element per core, weights
replicated, no collectives.

Per-core layout (all feature-major, zero on-device transposes); matmul
operands stored in bf16 (fp32 PSUM accumulation), fp32 output:
  host sends xT = x[b].T  [d, t]  and W*T = W*.T  [d_in, d_out]
  qT[do,t] = sum_d WqT[d,do] * xT[d,t]   (lhsT=WqT, rhs=xT, moving dim 1024)
  kT       likewise
  v[t,do]  = sum_d xT[d,t]  * WvT[d,do]  (lhsT=xT,  rhs=WvT) -> natural layout
  per head pair (2p, 2p+1), per tq block of 512, per tk block of 128:
    S^T[tk,tq]   = sum_hd kT_h[hd,tk] qT_h[hd,tq]   (both heads into one
                   [128,1024] PSUM superblock, one 512-col half per head)
    E = exp(S^T/8)                                  (one ACT op per superblock)
    diag blocks:  E = affine_select(E, f - p - 128g >= 0, fill=0) on GpSimd
                  (causal mask as a zero-fill, no PE mask matmuls)
    outX[m,tq]   = sum_tk Vplus_h[tk,m] E_h[tk,tq], Vplus_h = [v_h | 1]
    outT_h       = outX[0:64] * recip(outX[64])     (flash-style denominator;
                   reciprocal_approx_fast batched 4 rows at a time, broadcast
                   across partitions via GpSimd partition_broadcast)
  y[t,do] = sum_d outT[d,t] * WoT[d,do]  (lhsT=outT, rhs=WoT)
"""

import numpy as np
from contextlib import ExitStack

N_CORES = 8
T = 1024
D = 1024
NH = 16
HD = 64  # head dim
P = 128
NT = D // P  # 8 tiles of 128 along d or t
NQ = 2       # tq tiles of 512
TQ = 512
SCALE = 1.0 / 8.0  # 1/sqrt(64)

# matmul compute dtype: "fp32" (exact, 1/4 rate), "fp32r" (TF32-like, full
# rate), or "bf16" (full rate, half DMA/SBUF traffic)
MM_MODE = "bf16"

_COMPILED = {}


def _build(nc, tile, mybir, mm_dt):
    """Emit the per-core Tile program into nc."""
    f32 = mybir.dt.float32
    Exp = mybir.ActivationFunctionType.Exp

    mdt = mm_dt  # storage dtype for matmul operands
    xT_d = nc.declare_dram_parameter("xT", [D, T], mdt, isOutput=False)
    wqT_d = nc.declare_dram_parameter("wqT", [D, D], mdt, isOutput=False)
    wkT_d = nc.declare_dram_parameter("wkT", [D, D], mdt, isOutput=False)
    wvT_d = nc.declare_dram_parameter("wvT", [D, D], mdt, isOutput=False)
    woT_d = nc.declare_dram_parameter("woT", [D, D], mdt, isOutput=False)
    y_d = nc.declare_dram_parameter("y", [T, D], f32, isOutput=True)

    nrm_d = nc.dram_tensor("nrm_scratch", [8, 4, TQ], f32)

    def mm(out, lhsT, rhs, start, stop):
        nc.tensor.matmul(out, lhsT, rhs, start=start, stop=stop)

    with ExitStack() as ctx:
        tc = ctx.enter_context(tile.TileContext(nc))

        # ---- resident pools ----
        pqk = ctx.enter_context(tc.tile_pool(name="pqk", bufs=16))
        pv = ctx.enter_context(tc.tile_pool(name="pv", bufs=8))
        pwo = ctx.enter_context(tc.tile_pool(name="pwo", bufs=1))
        # psum: 3 superblock slots (2 banks each) + 2 attV slots = 8 banks
        pp_big = ctx.enter_context(
            tc.tile_pool(name="pp_big", bufs=3, space="PSUM")
        )
        pp_o = ctx.enter_context(tc.tile_pool(name="pp_o", bufs=2, space="PSUM"))

        qT = [pqk.tile([P, T], mdt, tag="qk", name=f"qT{i}") for i in range(NT)]
        kT = [pqk.tile([P, T], mdt, tag="qk", name=f"kT{i}") for i in range(NT)]
        # v-plus layout: head h at cols [65h .. 65h+63], ones col at 65h+64
        VP = HD + 1
        vp = [pv.tile([P, NH * VP], mdt, tag="vp", name=f"vp{i}") for i in range(NT)]
        wo_all = pwo.tile([P, NT * D], mdt, name="wo_all")

        # ---- phase 1: projections ----
        with (
            tc.tile_pool(name="pwq", bufs=8) as pwq,
            tc.tile_pool(name="pwkv", bufs=2) as pwkv,
            tc.tile_pool(name="pxt", bufs=8) as pxt,
        ):
            xT = [pxt.tile([P, T], mdt, tag="xt", name=f"xTs{i}")
                  for i in range(NT)]
            wq_sb = [pwq.tile([P, D], mdt, tag="wq", name=f"wq{i}")
                     for i in range(NT)]
            # dependency-first DMA order: wq/xT per-k interleaved so the
            # first matmul only waits on the first two transfers
            for k in range(NT):
                nc.sync.dma_start(
                    out=wq_sb[k][:], in_=wqT_d[k * P : (k + 1) * P, :]
                )
                nc.sync.dma_start(
                    out=xT[k][:], in_=xT_d[k * P : (k + 1) * P, :]
                )
            wk_all = pwkv.tile([P, NT * D], mdt, tag="wkv", name="wk_all")
            wv_all = pwkv.tile([P, NT * D], mdt, tag="wkv", name="wv_all")
            nc.sync.dma_start(
                out=wk_all.rearrange("p (k d) -> p k d", d=D),
                in_=wkT_d.rearrange("(k p) d -> p k d", p=P),
            )
            nc.sync.dma_start(
                out=wv_all.rearrange("p (k d) -> p k d", d=D),
                in_=wvT_d.rearrange("(k p) d -> p k d", p=P),
            )
            nc.sync.dma_start(
                out=wo_all.rearrange("p (k d) -> p k d", d=D),
                in_=woT_d.rearrange("(k p) d -> p k d", p=P),
            )
            # ones columns of v-plus (denominator rows for the attV matmul)
            for m in range(NT):
                ones_cols = vp[m].rearrange(
                    "p (h c) -> p h c", c=VP
                )[:, :, VP - 1]
                nc.gpsimd.memset(ones_cols, 1.0)

            for m in range(NT):  # q projection
                ps = pp_big.tile([P, T], f32, tag="ps")
                for n in range(NQ):
                    for k in range(NT):
                        mm(ps[:, n * TQ : (n + 1) * TQ],
                           wq_sb[k][:, m * P : (m + 1) * P],
                           xT[k][:, n * TQ : (n + 1) * TQ],
                           start=(k == 0), stop=(k == NT - 1))
                nc.vector.tensor_copy(qT[m][:], ps[:])
            for m in range(NT):  # k projection
                ps = pp_big.tile([P, T], f32, tag="ps")
                for n in range(NQ):
                    for k in range(NT):
                        mm(ps[:, n * TQ : (n + 1) * TQ],
                           wk_all[:, k * D + m * P : k * D + (m + 1) * P],
                           xT[k][:, n * TQ : (n + 1) * TQ],
                           start=(k == 0), stop=(k == NT - 1))
                nc.vector.tensor_copy(kT[m][:], ps[:])
            for m in range(NT):  # v projection (natural [t, do] layout)
                ps = pp_big.tile([P, T], f32, tag="ps")
                for n in range(NQ):
                    for k in range(NT):
                        mm(ps[:, n * TQ : (n + 1) * TQ],
                           xT[k][:, m * P : (m + 1) * P],
                           wv_all[:, k * D + n * TQ : k * D + (n + 1) * TQ],
                           start=(k == 0), stop=(k == NT - 1))
                vdst = vp[m].rearrange("p (h c) -> p h c", c=VP)[:, :, 0:HD]
                vsrc = ps.rearrange("p (h c) -> p h c", c=HD)
                nc.vector.tensor_copy(vdst, vsrc)

        # outT opens after phase-1 pools close (stack alloc reuses space)
        pout = ctx.enter_context(tc.tile_pool(name="pout", bufs=8))
        outT = [pout.tile([P, T], mdt, tag="ot", name=f"outT{i}")
                for i in range(NT)]

        # ---- phase 2: attention ----
        with (
            tc.tile_pool(name="pe", bufs=12) as pe,
            tc.tile_pool(name="pux", bufs=6) as pux,
            tc.tile_pool(name="pct", bufs=2) as pct,
            tc.tile_pool(name="pbc", bufs=4) as pbc,
        ):
            pending = []  # (ux, it, j, half) awaiting normalize
            state = {"flush_no": 0}

            def flush():
                # denominator rows sit at partition 64 of each ux; gather
                # them to partitions 0..3, one batched fast reciprocal, then
                # broadcast across partitions via a DRAM-bounce DMA
                if not pending:
                    return
                fno = state["flush_no"]
                nb = len(pending)
                ct = pct.tile([4, TQ], f32, tag="ct")
                cr = pct.tile([4, TQ], f32, tag="cr")
                for r, (ux, _, _, _) in enumerate(pending):
                    nc.sync.dma_start(
                        out=ct[r : r + 1, :], in_=ux[HD : HD + 1, :]
                    )
                nc.vector.reciprocal_approx_fast(cr[:nb, :], ct[:nb, :])
                nc.sync.dma_start(out=nrm_d[fno, :nb, :], in_=cr[:nb, :])
                for r, (ux, it, j, half) in enumerate(pending):
                    bt = pbc.tile([HD, TQ], f32, tag="bt")
                    nc.sync.dma_start(
                        out=bt[:],
                        in_=nrm_d[fno, r : r + 1, :].to_broadcast([HD, TQ]),
                    )
                    if half == 0:
                        nc.vector.tensor_mul(
                            outT[it][0:HD, j * TQ : (j + 1) * TQ],
                            ux[0:HD, :], bt[:],
                        )
                    else:
                        nt_ = pbc.tile([HD, TQ], mdt, tag="nt")
                        nc.vector.tensor_mul(nt_[:], ux[0:HD, :], bt[:])
                        nc.sync.dma_start(
                            out=outT[it][HD:P, j * TQ : (j + 1) * TQ],
                            in_=nt_[:],
                        )
                pending.clear()
                state["flush_no"] = (fno + 1) % 8

            for pr in range(NH // 2):
                it = pr  # qT/kT/outT tile index for this pair
                for j in range(NQ):
                    ni = 4 * j + 4  # tk tiles needed: i < ni
                    esup = []
                    for i in range(ni):
                        ps = pp_big.tile([P, 2 * TQ], f32, tag="ps")
                        g = i - 4 * j
                        for half in range(2):
                            po = half * HD
                            c = half * TQ
                            mm(ps[:, c : c + TQ],
                               kT[it][po : po + HD, i * P : (i + 1) * P],
                               qT[it][po : po + HD, j * TQ : (j + 1) * TQ],
                               start=True, stop=True)
                        e = pe.tile([P, 2 * TQ], mdt, tag="e")
                        nc.scalar.activation(e[:], ps[:], Exp, scale=SCALE)
                        if g >= 0:  # causal mask: keep where f - p - 128g >= 0
                            ev = e.rearrange("p (h f) -> p h f", h=2)
                            nc.gpsimd.affine_select(
                                ev, ev,
                                pattern=[[0, 2], [1, TQ]],
                                compare_op=mybir.AluOpType.is_ge,
                                fill=0.0,
                                base=-(P * g),
                                channel_multiplier=-1,
                            )
                        esup.append(e)
                    for half in range(2):
                        h = 2 * pr + half
                        po_ps = pp_o.tile([HD + 1, TQ], f32)
                        for i in range(ni):
                            mm(po_ps[:],
                               vp[i][:, h * VP : h * VP + VP],
                               esup[i][:, half * TQ : (half + 1) * TQ],
                               start=(i == 0), stop=(i == ni - 1))
                        ux = pux.tile([HD + 1, TQ], f32, tag="ux")
                        nc.vector.tensor_copy(ux[:], po_ps[:])
                        pending.append((ux, it, j, half))
                flush()

        # ---- phase 3: output projection ----
        # first NPRE column blocks accumulate k=0..6 early so only the k=7
        # term waits on the last attention pair's outT[7]
        NPRE = 3
        with tc.tile_pool(name="py", bufs=3) as py:
            pre = []
            for m in range(NPRE):
                ps = pp_big.tile([P, T], f32, tag="ps")
                for n in range(NQ):
                    for k in range(NT - 1):
                        mm(ps[:, n * TQ : (n + 1) * TQ],
                           outT[k][:, m * P : (m + 1) * P],
                           wo_all[:, k * D + n * TQ : k * D + (n + 1) * TQ],
                           start=(k == 0), stop=False)
                pre.append(ps)
            for m in range(NT):
                if m < NPRE:
                    ps = pre[m]
                    k0 = NT - 1
                else:
                    ps = pp_big.tile([P, T], f32, tag="ps")
                    k0 = 0
                for n in range(NQ):
                    for k in range(k0, NT):
                        mm(ps[:, n * TQ : (n + 1) * TQ],
                           outT[k][:, m * P : (m + 1) * P],
                           wo_all[:, k * D + n * TQ : k * D + (n + 1) * TQ],
                           start=(k == 0), stop=(k == NT - 1))
                ysb = py.tile([P, T], f32, tag="y")
                nc.vector.tensor_copy(ysb[:], ps[:])
                nc.sync.dma_start(
                    out=y_d[m * P : (m + 1) * P, :], in_=ysb[:]
                )
    return nc


def build_program(mm_mode=None):
    """Build + compile the SPMD program once; returns the Bacc object."""
    mode = mm_mode or MM_MODE
    if mode in _COMPILED:
        return _COMPILED[mode]
    import concourse.bacc as bacc
    import concourse.tile as tile
    from concourse import mybir

    mm_dt = {
        "fp32": mybir.dt.float32,
        "fp32r": mybir.dt.float32r,
        "bf16": mybir.dt.bfloat16,
    }[mode]
    nc = bacc.Bacc("TRN2", target_bir_lowering=False, debug=False,
                   num_devices=N_CORES)
    _build(nc, tile, mybir, mm_dt)
    nc.compile()
    _COMPILED[mode] = nc
    return nc


def _np_dt():
    if MM_MODE == "bf16":
        import ml_dtypes
        return ml_dtypes.bfloat16
    return np.float32


def make_in_maps(x, Wk, Wq, Wv, Wo):
    dt = _np_dt()
    wqT = np.ascontiguousarray(np.asarray(Wq, dtype=np.float32).T.astype(dt))
    wkT = np.ascontiguousarray(np.asarray(Wk, dtype=np.float32).T.astype(dt))
    wvT = np.ascontiguousarray(np.asarray(Wv, dtype=np.float32).T.astype(dt))
    woT = np.ascontiguousarray(np.asarray(Wo, dtype=np.float32).T.astype(dt))
    in_maps = []
    for b in range(N_CORES):
        in_maps.append({
            "xT": np.ascontiguousarray(x[b].T.astype(dt)),
            "wqT": wqT, "wkT": wkT, "wvT": wvT, "woT": woT,
        })
    return in_maps


def kernel(x, Wk, Wq, Wv, Wo):
    from concourse.bass_utils import run_bass_kernel_spmd

    x = np.asarray(x, dtype=np.float32)
    nc = build_program()
    in_maps = make_in_maps(x, Wk, Wq, Wv, Wo)
    res = run_bass_kernel_spmd(nc, in_maps, list(range(N_CORES)))
    return np.stack([res.results[c]["y"] for c in range(N_CORES)], axis=0)


# revision 16
# speedup vs baseline: 1.2080x; 1.0512x over previous
"""Causal multi-head self-attention on 8 Trainium2 NeuronCores.

Problem: B=8, T=1024, D=1024, 16 heads (H=64), fp32 in/out, causal softmax,
y = softmax(mask(q k^T)/sqrt(H)) v, then output projection. Weights are
nn.Linear style: q = x @ Wq^T etc.

Sharding: pure data-parallel - one batch element per core, weights
replicated, no collectives.

Per-core layout (all feature-major, zero on-device transposes); matmul
operands stored in bf16 (fp32 PSUM accumulation), fp32 output:
  host sends xT = x[b].T  [d, t]  and W*T = W*.T  [d_in, d_out]
  qT[do,t] = sum_d WqT[d,do] * xT[d,t]   (lhsT=WqT, rhs=xT, moving dim 1024)
  kT       likewise
  v[t,do]  = sum_d xT[d,t]  * WvT[d,do]  (lhsT=xT,  rhs=WvT) -> natural layout
  per head pair (2p, 2p+1), per tq block of 512, per tk block of 128:
    S^T[tk,tq]   = sum_hd kT_h[hd,tk] qT_h[hd,tq]   (both heads into one
                   [128,1024] PSUM superblock, one 512-col half per head)
    E = exp(S^T/8)                                  (one ACT op per superblock)
    diag blocks:  E = affine_select(E, f - p - 128g >= 0, fill=0) on GpSimd
                  (causal mask as a zero-fill, no PE mask matmuls)
    outX[m,tq]   = sum_tk Vplus_h[tk,m] E_h[tk,tq], Vplus_h = [v_h | 1]
    outT_h       = outX[0:64] * recip(outX[64])     (flash-style denominator;
                   reciprocal_approx_fast batched 4 rows at a time, broadcast
                   across partitions via GpSimd partition_broadcast)
  y[t,do] = sum_d outT[d,t] * WoT[d,do]  (lhsT=outT, rhs=WoT)
"""

import numpy as np
from contextlib import ExitStack

N_CORES = 8
T = 1024
D = 1024
NH = 16
HD = 64  # head dim
P = 128
NT = D // P  # 8 tiles of 128 along d or t
NQ = 2       # tq tiles of 512
TQ = 512
SCALE = 1.0 / 8.0  # 1/sqrt(64)

# matmul compute dtype: "fp32" (exact, 1/4 rate), "fp32r" (TF32-like, full
# rate), or "bf16" (full rate, half DMA/SBUF traffic)
MM_MODE = "bf16"

_COMPILED = {}


def _build(nc, tile, mybir, mm_dt):
    """Emit the per-core Tile program into nc."""
    f32 = mybir.dt.float32
    Exp = mybir.ActivationFunctionType.Exp

    mdt = mm_dt  # storage dtype for matmul operands
    xT_d = nc.declare_dram_parameter("xT", [D, T], mdt, isOutput=False)
    wqT_d = nc.declare_dram_parameter("wqT", [D, D], mdt, isOutput=False)
    wkT_d = nc.declare_dram_parameter("wkT", [D, D], mdt, isOutput=False)
    wvT_d = nc.declare_dram_parameter("wvT", [D, D], mdt, isOutput=False)
    woT_d = nc.declare_dram_parameter("woT", [D, D], mdt, isOutput=False)
    y_d = nc.declare_dram_parameter("y", [T, D], f32, isOutput=True)

    nrm_d = nc.dram_tensor("nrm_scratch", [16, 4, TQ], f32)

    def mm(out, lhsT, rhs, start, stop):
        nc.tensor.matmul(out, lhsT, rhs, start=start, stop=stop)

    with ExitStack() as ctx:
        tc = ctx.enter_context(tile.TileContext(nc))

        # ---- resident pools ----
        pqk = ctx.enter_context(tc.tile_pool(name="pqk", bufs=16))
        pv = ctx.enter_context(tc.tile_pool(name="pv", bufs=8))
        pwo = ctx.enter_context(tc.tile_pool(name="pwo", bufs=1))
        # psum: 3 superblock slots (2 banks each) + 2 attV slots = 8 banks
        pp_big = ctx.enter_context(
            tc.tile_pool(name="pp_big", bufs=3, space="PSUM")
        )
        pp_o = ctx.enter_context(tc.tile_pool(name="pp_o", bufs=2, space="PSUM"))

        qT = [pqk.tile([P, T], mdt, tag="qk", name=f"qT{i}") for i in range(NT)]
        kT = [pqk.tile([P, T], mdt, tag="qk", name=f"kT{i}") for i in range(NT)]
        # v-plus layout: head h at cols [65h .. 65h+63], ones col at 65h+64
        VP = HD + 1
        vp = [pv.tile([P, NH * VP], mdt, tag="vp", name=f"vp{i}") for i in range(NT)]
        wo_all = pwo.tile([P, NT * D], mdt, name="wo_all")

        # ---- phase 1: projections ----
        with (
            tc.tile_pool(name="pwq", bufs=8) as pwq,
            tc.tile_pool(name="pwkv", bufs=2) as pwkv,
            tc.tile_pool(name="pxt", bufs=8) as pxt,
        ):
            xT = [pxt.tile([P, T], mdt, tag="xt", name=f"xTs{i}")
                  for i in range(NT)]
            wq_sb = [pwq.tile([P, D], mdt, tag="wq", name=f"wq{i}")
                     for i in range(NT)]
            # dependency-first DMA order: wq/xT per-k interleaved so the
            # first matmul only waits on the first two transfers
            for k in range(NT):
                nc.sync.dma_start(
                    out=wq_sb[k][:], in_=wqT_d[k * P : (k + 1) * P, :]
                )
                nc.sync.dma_start(
                    out=xT[k][:], in_=xT_d[k * P : (k + 1) * P, :]
                )
            wk_all = pwkv.tile([P, NT * D], mdt, tag="wkv", name="wk_all")
            wv_all = pwkv.tile([P, NT * D], mdt, tag="wkv", name="wv_all")
            nc.sync.dma_start(
                out=wk_all.rearrange("p (k d) -> p k d", d=D),
                in_=wkT_d.rearrange("(k p) d -> p k d", p=P),
            )
            nc.sync.dma_start(
                out=wv_all.rearrange("p (k d) -> p k d", d=D),
                in_=wvT_d.rearrange("(k p) d -> p k d", p=P),
            )
            nc.sync.dma_start(
                out=wo_all.rearrange("p (k d) -> p k d", d=D),
                in_=woT_d.rearrange("(k p) d -> p k d", p=P),
            )
            # ones columns of v-plus (denominator rows for the attV matmul)
            for m in range(NT):
                ones_cols = vp[m].rearrange(
                    "p (h c) -> p h c", c=VP
                )[:, :, VP - 1]
                nc.gpsimd.memset(ones_cols, 1.0)

            for m in range(NT):  # q projection
                ps = pp_big.tile([P, T], f32, tag="ps")
                for n in range(NQ):
                    for k in range(NT):
                        mm(ps[:, n * TQ : (n + 1) * TQ],
                           wq_sb[k][:, m * P : (m + 1) * P],
                           xT[k][:, n * TQ : (n + 1) * TQ],
                           start=(k == 0), stop=(k == NT - 1))
                nc.vector.tensor_copy(qT[m][:], ps[:])
            for m in range(NT):  # k projection
                ps = pp_big.tile([P, T], f32, tag="ps")
                for n in range(NQ):
                    for k in range(NT):
                        mm(ps[:, n * TQ : (n + 1) * TQ],
                           wk_all[:, k * D + m * P : k * D + (m + 1) * P],
                           xT[k][:, n * TQ : (n + 1) * TQ],
                           start=(k == 0), stop=(k == NT - 1))
                nc.vector.tensor_copy(kT[m][:], ps[:])
            for m in range(NT):  # v projection (natural [t, do] layout)
                ps = pp_big.tile([P, T], f32, tag="ps")
                for n in range(NQ):
                    for k in range(NT):
                        mm(ps[:, n * TQ : (n + 1) * TQ],
                           xT[k][:, m * P : (m + 1) * P],
                           wv_all[:, k * D + n * TQ : k * D + (n + 1) * TQ],
                           start=(k == 0), stop=(k == NT - 1))
                vdst = vp[m].rearrange("p (h c) -> p h c", c=VP)[:, :, 0:HD]
                vsrc = ps.rearrange("p (h c) -> p h c", c=HD)
                nc.vector.tensor_copy(vdst, vsrc)

        # outT opens after phase-1 pools close (stack alloc reuses space)
        pout = ctx.enter_context(tc.tile_pool(name="pout", bufs=8))
        outT = [pout.tile([P, T], mdt, tag="ot", name=f"outT{i}")
                for i in range(NT)]

        # ---- phase 2: attention ----
        # 0/1 keep-mask tiles for g=0,2 (generated on device): the causal
        # masking alternates between GpSimd affine_select and DVE multiply
        # so two diagonal tiles can be masked in parallel
        pkm = ctx.enter_context(tc.tile_pool(name="pkm", bufs=2))
        keepg = {}
        for g in (0, 2):
            km = pkm.tile([P, 2 * TQ], mdt, tag="km", name=f"keep{g}")
            nc.gpsimd.memset(km[:], 1.0)
            kv = km.rearrange("p (h f) -> p h f", h=2)
            nc.gpsimd.affine_select(
                kv, kv, pattern=[[0, 2], [1, TQ]],
                compare_op=mybir.AluOpType.is_ge, fill=0.0,
                base=-(P * g), channel_multiplier=-1,
            )
            keepg[g] = km
        with (
            tc.tile_pool(name="pe", bufs=12) as pe,
            tc.tile_pool(name="pux", bufs=6) as pux,
            tc.tile_pool(name="pct", bufs=2) as pct,
            tc.tile_pool(name="pbc", bufs=4) as pbc,
        ):
            pending = []  # (ux, it, j, half) awaiting normalize
            state = {"flush_no": 0}

            def flush():
                # denominator rows sit at partition 64 of each ux; gather
                # them to partitions 0..3, one batched fast reciprocal, then
                # broadcast across partitions via a DRAM-bounce DMA
                if not pending:
                    return
                fno = state["flush_no"]
                nb = len(pending)
                ct = pct.tile([4, TQ], f32, tag="ct")
                cr = pct.tile([4, TQ], f32, tag="cr")
                for r, (ux, _, _, _) in enumerate(pending):
                    nc.sync.dma_start(
                        out=ct[r : r + 1, :], in_=ux[HD : HD + 1, :]
                    )
                nc.vector.reciprocal_approx_fast(cr[:nb, :], ct[:nb, :])
                nc.sync.dma_start(out=nrm_d[fno, :nb, :], in_=cr[:nb, :])
                for r, (ux, it, j, half) in enumerate(pending):
                    bt = pbc.tile([HD, TQ], f32, tag="bt")
                    nc.sync.dma_start(
                        out=bt[:],
                        in_=nrm_d[fno, r : r + 1, :].to_broadcast([HD, TQ]),
                    )
                    if half == 0:
                        nc.vector.tensor_mul(
                            outT[it][0:HD, j * TQ : (j + 1) * TQ],
                            ux[0:HD, :], bt[:],
                        )
                    else:
                        nt_ = pbc.tile([HD, TQ], mdt, tag="nt")
                        nc.vector.tensor_mul(nt_[:], ux[0:HD, :], bt[:])
                        nc.sync.dma_start(
                            out=outT[it][HD:P, j * TQ : (j + 1) * TQ],
                            in_=nt_[:],
                        )
                pending.clear()
                state["flush_no"] = (fno + 1) % 16

            for pr in range(NH // 2):
                it = pr  # qT/kT/outT tile index for this pair
                for j in range(NQ):
                    ni = 4 * j + 4  # tk tiles needed: i < ni
                    # descending order: diagonal tiles (largest i) first, so
                    # their exp+mask latency hides under the remaining scores
                    order = list(range(ni - 1, -1, -1))
                    esup = {}
                    for i in order:
                        ps = pp_big.tile([P, 2 * TQ], f32, tag="ps")
                        g = i - 4 * j
                        for half in range(2):
                            po = half * HD
                            c = half * TQ
                            mm(ps[:, c : c + TQ],
                               kT[it][po : po + HD, i * P : (i + 1) * P],
                               qT[it][po : po + HD, j * TQ : (j + 1) * TQ],
                               start=True, stop=True)
                        e = pe.tile([P, 2 * TQ], mdt, tag="e")
                        nc.scalar.activation(e[:], ps[:], Exp, scale=SCALE)
                        if g >= 0:  # causal mask: keep where f - p - 128g >= 0
                            if g in keepg:  # DVE multiply path
                                nc.vector.tensor_mul(e[:], e[:], keepg[g][:])
                            else:  # GpSimd affine_select path
                                ev = e.rearrange("p (h f) -> p h f", h=2)
                                nc.gpsimd.affine_select(
                                    ev, ev,
                                    pattern=[[0, 2], [1, TQ]],
                                    compare_op=mybir.AluOpType.is_ge,
                                    fill=0.0,
                                    base=-(P * g),
                                    channel_multiplier=-1,
                                )
                        esup[i] = e
                    for half in range(2):
                        h = 2 * pr + half
                        po_ps = pp_o.tile([HD + 1, TQ], f32)
                        for ii, i in enumerate(order):
                            mm(po_ps[:],
                               vp[i][:, h * VP : h * VP + VP],
                               esup[i][:, half * TQ : (half + 1) * TQ],
                               start=(ii == 0), stop=(ii == ni - 1))
                        ux = pux.tile([HD + 1, TQ], f32, tag="ux")
                        nc.vector.tensor_copy(ux[:], po_ps[:])
                        pending.append((ux, it, j, half))
                    if pr == NH // 2 - 1:
                        # last pair: flush per j so the output projection's
                        # first column blocks only wait on the j=0 half
                        flush()
                flush()

        # ---- phase 3: output projection ----
        # first NPRE column blocks accumulate k=0..6 early so only the k=7
        # term waits on the last attention pair's outT[7]
        NPRE = 3
        with tc.tile_pool(name="py", bufs=3) as py:
            pre = []
            for m in range(NPRE):
                ps = pp_big.tile([P, T], f32, tag="ps")
                for n in range(NQ):
                    for k in range(NT - 1):
                        mm(ps[:, n * TQ : (n + 1) * TQ],
                           outT[k][:, m * P : (m + 1) * P],
                           wo_all[:, k * D + n * TQ : k * D + (n + 1) * TQ],
                           start=(k == 0), stop=False)
                pre.append(ps)
            for m in range(NT):
                if m < NPRE:
                    ps = pre[m]
                    k0 = NT - 1
                else:
                    ps = pp_big.tile([P, T], f32, tag="ps")
                    k0 = 0
                for n in range(NQ):
                    for k in range(k0, NT):
                        mm(ps[:, n * TQ : (n + 1) * TQ],
                           outT[k][:, m * P : (m + 1) * P],
                           wo_all[:, k * D + n * TQ : k * D + (n + 1) * TQ],
                           start=(k == 0), stop=(k == NT - 1))
                ysb = py.tile([P, T], f32, tag="y")
                nc.vector.tensor_copy(ysb[:], ps[:])
                nc.sync.dma_start(
                    out=y_d[m * P : (m + 1) * P, :], in_=ysb[:]
                )
    return nc


def build_program(mm_mode=None):
    """Build + compile the SPMD program once; returns the Bacc object."""
    mode = mm_mode or MM_MODE
    if mode in _COMPILED:
        return _COMPILED[mode]
    import concourse.bacc as bacc
    import concourse.tile as tile
    from concourse import mybir

    mm_dt = {
        "fp32": mybir.dt.float32,
        "fp32r": mybir.dt.float32r,
        "bf16": mybir.dt.bfloat16,
    }[mode]
    nc = bacc.Bacc("TRN2", target_bir_lowering=False, debug=False,
                   num_devices=N_CORES)
    _build(nc, tile, mybir, mm_dt)
    nc.compile()
    _COMPILED[mode] = nc
    return nc


def _np_dt():
    if MM_MODE == "bf16":
        import ml_dtypes
        return ml_dtypes.bfloat16
    return np.float32


def make_in_maps(x, Wk, Wq, Wv, Wo):
    dt = _np_dt()
    wqT = np.ascontiguousarray(np.asarray(Wq, dtype=np.float32).T.astype(dt))
    wkT = np.ascontiguousarray(np.asarray(Wk, dtype=np.float32).T.astype(dt))
    wvT = np.ascontiguousarray(np.asarray(Wv, dtype=np.float32).T.astype(dt))
    woT = np.ascontiguousarray(np.asarray(Wo, dtype=np.float32).T.astype(dt))
    in_maps = []
    for b in range(N_CORES):
        in_maps.append({
            "xT": np.ascontiguousarray(x[b].T.astype(dt)),
            "wqT": wqT, "wkT": wkT, "wvT": wvT, "woT": woT,
        })
    return in_maps


def kernel(x, Wk, Wq, Wv, Wo):
    from concourse.bass_utils import run_bass_kernel_spmd

    x = np.asarray(x, dtype=np.float32)
    nc = build_program()
    in_maps = make_in_maps(x, Wk, Wq, Wv, Wo)
    res = run_bass_kernel_spmd(nc, in_maps, list(range(N_CORES)))
    return np.stack([res.results[c]["y"] for c in range(N_CORES)], axis=0)
